# revision 37
# baseline (speedup 1.0000x reference)
"""Trainium2 Bass kernel for BaseSpectrogram1D.

x[128, 131072] -> |DFT(window * overlapping_frames(x - mean))| [128, 511, 257]

Sharding: pure data parallel, batch dim split across 8 NeuronCores
(16 rows each). window/fourier_matrix are combined host-side into one
real [512, 512] fp16 matrix (Re bins 0..256 | Im bins 1..255; Im[0] and
Im[256] are exactly zero and omitted so a frame-tile's whole DFT fits a
single 512-wide PSUM bank).

x is cast to fp16 and pre-transposed host-side (fp16 matmuls get fast
separate weight loads, unlike fp32/fp32r whose in-matmul 4-byte weight
load costs as much as the matmul itself; the transposed layout puts
the contraction dim on SBUF partitions).

Structure per core:
  phase 0: plain loads + DVE row-sums + per-batch ones-matmul/ACT
    scale -> neg-mean broadcast to every partition.
  phase 1, per batch: DVE mean-subtract (fp16), then m-tile mt
    computes frames {4p+mt} via 4 accumulated fp16 matmuls (stride-8
    column slices of the transposed x are exactly frames^T) -> PSUM
    [128, 512]; frame 511 is junk padding (127-wide weights would
    disable the fast weight load). Squares from PSUM (ACT Square on
    even tiles, DVE copy+mul on odd ones), GpSimd pairs re^2+im^2
    (+ copies bins 0/256), ACT Sqrt into a per-group accumulator.
    Every 8 batches, 4 batch-striped stores (1028B descriptors whose
    stream steps 525KB through DRAM spread across the SDMA engines;
    dense single-batch stores run on ONE engine at ~27 GB/s).
"""

import sys

if "/opt/trn_rl_repo" not in sys.path:
    sys.path.insert(0, "/opt/trn_rl_repo")

import numpy as np

L = 131072
B = 128
N = 512  # frame length
M = 511  # frames
STRIDE = 256
KH = 257  # one-sided bins
NCORES = 8
BPC = B // NCORES  # batches per core = 16

_CACHE = {}


def _tukey(n_pts, alpha=0.25):
    n = np.arange(n_pts, dtype=np.float64)
    edge = alpha * (n_pts - 1) / 2.0
    w = np.ones(n_pts)
    left = n < edge
    w[left] = 0.5 * (1.0 + np.cos(np.pi * (2.0 * n[left] / (alpha * (n_pts - 1)) - 1.0)))
    right = n > (n_pts - 1) - edge
    w[right] = 0.5 * (
        1.0 + np.cos(np.pi * (2.0 * n[right] / (alpha * (n_pts - 1)) - 2.0 / alpha + 1.0))
    )
    return w


def _default_consts():
    w = _tukey(N, 0.25)
    w = (w / w.sum()).astype(np.float32)
    nk = np.outer(np.arange(N, dtype=np.float64), np.arange(N, dtype=np.float64))
    sigma = np.exp(-2j * np.pi / N)
    fm = (sigma**nk)[:, :KH] * np.sqrt(N)
    return w, fm.astype(np.complex64)


def _build():
    """Build + schedule the Bass module once per process."""
    if "nc" in _CACHE:
        return _CACHE["nc"]

    import concourse.mybir as mybir
    import concourse.tile as tile
    from concourse import bacc

    F32 = mybir.dt.float32
    F16 = mybir.dt.float16
    AF = mybir.ActivationFunctionType

    nc = bacc.Bacc(trn_type="TRN2", target_bir_lowering=False, debug=False)

    x_d = nc.dram_tensor("x", [BPC, 128, 1024], F16, kind="ExternalInput").ap()
    wfm_d = nc.dram_tensor("wfm", [4, 128, N], F16, kind="ExternalInput").ap()
    out_d = nc.dram_tensor("out", [BPC, M, KH], F32, kind="ExternalOutput").ap()

    with tile.TileContext(nc) as tc:
        with (
            tc.tile_pool(name="consts", bufs=1) as consts,
            tc.tile_pool(name="xtraw", bufs=3) as xtrawp,
            tc.tile_pool(name="small", bufs=4) as smallp,
            tc.tile_pool(name="xt", bufs=3) as xtp,
            tc.tile_pool(name="sq", bufs=6) as sqp,
            tc.tile_pool(name="mag", bufs=2) as magp,
            tc.tile_pool(name="msq", bufs=6) as msqp,
            tc.tile_pool(name="pmu", bufs=2, space="PSUM") as pmup,
            tc.tile_pool(name="pspec", bufs=6, space="PSUM") as pspecp,
        ):
            wfm_s = consts.tile([128, 4, N], F16)
            ones = consts.tile([128, 128], F32)
            nc.sync.dma_start(out=wfm_s, in_=wfm_d.rearrange("j p n -> p j n"))
            nc.vector.memset(ones, 1.0)

            # phase 0: all xbar transposes + per-batch means. Keeps every
            # transpose-mode DMA away from the copy-mode stores (each
            # xbar-mode transition serializes the DMA subsystem ~3.6us).
            xts, negmus = [], []
            for b in range(BPC):
                xt_raw = xtrawp.tile([128, 1024], F16, tag=f"xtr{b}")
                nc.sync.dma_start(out=xt_raw, in_=x_d[b])
                part = smallp.tile([128, 1], F32, tag="part")
                nc.vector.reduce_sum(part, xt_raw, axis=mybir.AxisListType.X)
                mu_ps = pmup.tile([128, 1], F32)
                nc.tensor.matmul(mu_ps, ones, part, start=True, stop=True)
                negmu = smallp.tile([128, 1], F32, tag=f"negmu{b}")
                nc.scalar.activation(negmu, mu_ps, AF.Copy, scale=-1.0 / L)
                xts.append(xt_raw)
                negmus.append(negmu)

            for b in range(BPC):
                xt_raw = xts[b]
                negmu = negmus[b]
                xt = xtp.tile([128, 1028], F16, tag="xt")
                nc.vector.memset(xt[:, 1024:1028], 0.0)
                nc.vector.tensor_scalar_add(xt[:, 0:1024], xt_raw, negmu)

                # m-tile mt covers frames {4p + mt} (chunk cols 8p + 2mt+j,
                # i.e. stride-8 lhsT slices). Output accumulates per m-tile
                # across GRP batches so store descriptors can stripe across
                # batches (525KB apart in DRAM -> many SDMA engines; a dense
                # single-batch store runs on ONE engine at ~27 GB/s).
                GRP = 8
                if b % GRP == 0:
                    magmt = [
                        magp.tile(
                            [128, GRP, KH],
                            F32,
                            tag=f"magmt{mt}",
                            name=f"magmt{mt}_{b}",
                        )
                        for mt in range(4)
                    ]
                for mt in range(4):
                    spec = pspecp.tile([128, N], F32)
                    for j in range(4):
                        r = 2 * mt + j  # 0..9
                        lhsT = xt[:, r : r + 1017 : 8]
                        nc.tensor.matmul(
                            spec,
                            lhsT,
                            wfm_s[:, j],
                            start=(j == 0),
                            stop=(j == 3),
                        )
                    # squares: alternate ACT / DVE to balance engines
                    sq = sqp.tile([128, N], F16, tag="sq")
                    if mt % 2 == 0:
                        nc.scalar.activation(sq, spec, AF.Square)
                    else:
                        sp16 = sqp.tile([128, N], F16, tag="sp16")
                        nc.vector.tensor_copy(sp16, spec)
                        nc.vector.tensor_mul(sq, sp16, sp16)
                    magsq = msqp.tile([128, KH], F16, tag="magsq")
                    nc.gpsimd.tensor_add(
                        magsq[:, 1:256], sq[:, 1:256], sq[:, 257:512]
                    )
                    nc.gpsimd.tensor_copy(magsq[:, 0:257:256], sq[:, 0:257:256])
                    nc.scalar.activation(magmt[mt][:, b % GRP], magsq, AF.Sqrt)
                if b % GRP == GRP - 1:
                    # batch-striped 1028B-descriptor stores on the sync ring
                    # (descriptor stream inner dim = g, 525KB apart in DRAM,
                    # spreads across the SDMA engines)
                    g0 = b - (GRP - 1)
                    for mt in range(4):
                        mm = 128 if mt < 3 else 127  # frame 511 is junk
                        # mt 0/2 (8B-aligned row starts) spread on the sync
                        # HWDGE ring; mt 1/3 hit its single-engine slow path,
                        # so route them through SWDGE which fans out to all
                        # 16 SDMA engines regardless of address.
                        eng = nc.sync if mt % 2 == 0 else nc.gpsimd
                        eng.dma_start(
                            out=out_d[
                                g0 : g0 + GRP, mt : mt + 4 * (mm - 1) + 1 : 4, :
                            ].rearrange("g m k -> m g k"),
                            in_=magmt[mt][:mm],
                        )

    nc.compile()
    _CACHE["nc"] = nc
    return nc


def make_inputs(x, window=None, fourier_matrix=None):
    """Host-side prep: fp16 cast/layout + combined DFT matrix."""
    x = np.asarray(x, dtype=np.float32)
    if window is None or fourier_matrix is None:
        window, fourier_matrix = _default_consts()
    window = np.asarray(window)
    fourier_matrix = np.asarray(fourier_matrix)

    wfm = fourier_matrix.astype(np.complex64) * window.astype(np.float32)[:, None]
    wfm_cat = np.concatenate(
        [wfm.real[:, 0:257], wfm.imag[:, 1:256]], axis=1
    ).astype(np.float16)  # [512, 512]
    wfm_in = np.ascontiguousarray(wfm_cat.reshape(4, 128, N))

    # pre-transposed: x16[b, e, c] = x[b, 128*c + e]
    x16 = np.ascontiguousarray(
        x.astype(np.float16).reshape(B, 1024, 128).transpose(0, 2, 1)
    )
    return x16, wfm_in


def kernel(x, window=None, fourier_matrix=None, **_unused):
    from concourse.bass_utils import run_bass_kernel_spmd

    x16, wfm_in = make_inputs(x, window, fourier_matrix)
    nc = _build()
    in_maps = [
        {"x": x16[i * BPC : (i + 1) * BPC], "wfm": wfm_in} for i in range(NCORES)
    ]
    res = run_bass_kernel_spmd(nc, in_maps, core_ids=list(range(NCORES)))
    return np.concatenate([r["out"] for r in res.results], axis=0)


if __name__ == "__main__":
    rng = np.random.default_rng(0)
    x = rng.standard_normal((B, L)).astype(np.float32)
    out = kernel(x)
    print("out", out.shape, out.dtype, float(out.max()))


# revision 39
# speedup vs baseline: 1.3044x; 1.3044x over previous
"""Trainium2 Bass kernel for BaseSpectrogram1D.

x[128, 131072] -> |DFT(window * overlapping_frames(x - mean))| [128, 511, 257]

Sharding: pure data parallel, batch dim split across 8 NeuronCores
(16 rows each). window/fourier_matrix are combined host-side into one
real [512, 512] fp16 matrix (Re bins 0..256 | Im bins 1..255; Im[0] and
Im[256] are exactly zero and omitted so a frame-tile's whole DFT fits a
single 512-wide PSUM bank).

x is cast to fp16 and pre-transposed host-side (fp16 matmuls get fast
separate weight loads, unlike fp32/fp32r whose in-matmul 4-byte weight
load costs as much as the matmul itself; the transposed layout puts
the contraction dim on SBUF partitions).

Structure per core:
  phase 0: plain loads + DVE row-sums + per-batch ones-matmul/ACT
    scale -> neg-mean broadcast to every partition.
  phase 1, per batch: DVE mean-subtract (fp16), then m-tile mt
    computes frames {4p+mt} via 4 accumulated fp16 matmuls (stride-8
    column slices of the transposed x are exactly frames^T) -> PSUM
    [128, 512]; frame 511 is junk padding (127-wide weights would
    disable the fast weight load). Squares from PSUM (ACT Square on
    even tiles, DVE copy+mul on odd ones), GpSimd pairs re^2+im^2
    (+ copies bins 0/256), ACT Sqrt into a per-group accumulator.
    Every 8 batches, 4 batch-striped stores (1028B descriptors whose
    stream steps 525KB through DRAM spread across the SDMA engines;
    dense single-batch stores run on ONE engine at ~27 GB/s).
"""

import sys

if "/opt/trn_rl_repo" not in sys.path:
    sys.path.insert(0, "/opt/trn_rl_repo")

import numpy as np

L = 131072
B = 128
N = 512  # frame length
M = 511  # frames
STRIDE = 256
KH = 257  # one-sided bins
NCORES = 8
BPC = B // NCORES  # batches per core = 16

_CACHE = {}


def _tukey(n_pts, alpha=0.25):
    n = np.arange(n_pts, dtype=np.float64)
    edge = alpha * (n_pts - 1) / 2.0
    w = np.ones(n_pts)
    left = n < edge
    w[left] = 0.5 * (1.0 + np.cos(np.pi * (2.0 * n[left] / (alpha * (n_pts - 1)) - 1.0)))
    right = n > (n_pts - 1) - edge
    w[right] = 0.5 * (
        1.0 + np.cos(np.pi * (2.0 * n[right] / (alpha * (n_pts - 1)) - 2.0 / alpha + 1.0))
    )
    return w


def _default_consts():
    w = _tukey(N, 0.25)
    w = (w / w.sum()).astype(np.float32)
    nk = np.outer(np.arange(N, dtype=np.float64), np.arange(N, dtype=np.float64))
    sigma = np.exp(-2j * np.pi / N)
    fm = (sigma**nk)[:, :KH] * np.sqrt(N)
    return w, fm.astype(np.complex64)


def _build():
    """Build + schedule the Bass module once per process."""
    if "nc" in _CACHE:
        return _CACHE["nc"]

    import concourse.mybir as mybir
    import concourse.tile as tile
    from concourse import bacc

    F32 = mybir.dt.float32
    F16 = mybir.dt.float16
    AF = mybir.ActivationFunctionType

    nc = bacc.Bacc(trn_type="TRN2", target_bir_lowering=False, debug=False)

    x_d = nc.dram_tensor("x", [BPC, 128, 1024], F16, kind="ExternalInput").ap()
    wfm_d = nc.dram_tensor("wfm", [4, 128, N], F16, kind="ExternalInput").ap()
    out_d = nc.dram_tensor("out", [BPC, M, KH], F32, kind="ExternalOutput").ap()

    with tile.TileContext(nc) as tc:
        with (
            tc.tile_pool(name="consts", bufs=1) as consts,
            tc.tile_pool(name="xtraw", bufs=3) as xtrawp,
            tc.tile_pool(name="small", bufs=4) as smallp,
            tc.tile_pool(name="xt", bufs=3) as xtp,
            tc.tile_pool(name="sq", bufs=6) as sqp,
            tc.tile_pool(name="mag", bufs=2) as magp,
            tc.tile_pool(name="msq", bufs=6) as msqp,
            tc.tile_pool(name="pmu", bufs=2, space="PSUM") as pmup,
            tc.tile_pool(name="pspec", bufs=6, space="PSUM") as pspecp,
        ):
            wfm_s = consts.tile([128, 4, N], F16)
            ones = consts.tile([128, 128], F32)
            nc.sync.dma_start(out=wfm_s, in_=wfm_d.rearrange("j p n -> p j n"))
            nc.vector.memset(ones, 1.0)

            # phase 0: all xbar transposes + per-batch means. Keeps every
            # transpose-mode DMA away from the copy-mode stores (each
            # xbar-mode transition serializes the DMA subsystem ~3.6us).
            xts, negmus = [], []
            for b in range(BPC):
                xt_raw = xtrawp.tile([128, 1024], F16, tag=f"xtr{b}")
                nc.sync.dma_start(out=xt_raw, in_=x_d[b])
                part = smallp.tile([128, 1], F32, tag="part")
                nc.vector.reduce_sum(part, xt_raw, axis=mybir.AxisListType.X)
                mu_ps = pmup.tile([128, 1], F32)
                nc.tensor.matmul(mu_ps, ones, part, start=True, stop=True)
                negmu = smallp.tile([128, 1], F32, tag=f"negmu{b}")
                nc.scalar.activation(negmu, mu_ps, AF.Copy, scale=-1.0 / L)
                xts.append(xt_raw)
                negmus.append(negmu)

            for b in range(BPC):
                xt_raw = xts[b]
                negmu = negmus[b]
                xt = xtp.tile([128, 1028], F16, tag="xt")
                nc.vector.memset(xt[:, 1024:1028], 0.0)
                nc.vector.tensor_scalar_add(xt[:, 0:1024], xt_raw, negmu)

                # m-tile mt covers frames {4p + mt} (chunk cols 8p + 2mt+j,
                # i.e. stride-8 lhsT slices). Output accumulates per m-tile
                # across GRP batches so store descriptors can stripe across
                # batches (525KB apart in DRAM -> many SDMA engines; a dense
                # single-batch store runs on ONE engine at ~27 GB/s).
                GRP = 8
                if b % GRP == 0:
                    magmt = [
                        magp.tile(
                            [128, GRP, KH],
                            F32,
                            tag=f"magmt{mt}",
                            name=f"magmt{mt}_{b}",
                        )
                        for mt in range(4)
                    ]
                for mt in range(4):
                    spec = pspecp.tile([128, N], F32)
                    for j in range(4):
                        r = 2 * mt + j  # 0..9
                        lhsT = xt[:, r : r + 1017 : 8]
                        nc.tensor.matmul(
                            spec,
                            lhsT,
                            wfm_s[:, j],
                            start=(j == 0),
                            stop=(j == 3),
                        )
                    # squares: alternate ACT / DVE to balance engines
                    sq = sqp.tile([128, N], F16, tag="sq")
                    if mt % 2 == 0:
                        nc.scalar.activation(sq, spec, AF.Square)
                    else:
                        sp16 = sqp.tile([128, N], F16, tag="sp16")
                        nc.vector.tensor_copy(sp16, spec)
                        nc.vector.tensor_mul(sq, sp16, sp16)
                    magsq = msqp.tile([128, KH], F16, tag="magsq")
                    nc.gpsimd.tensor_add(
                        magsq[:, 1:256], sq[:, 1:256], sq[:, 257:512]
                    )
                    nc.gpsimd.tensor_copy(magsq[:, 0:257:256], sq[:, 0:257:256])
                    nc.scalar.activation(magmt[mt][:, b % GRP], magsq, AF.Sqrt)
                if b % GRP == GRP // 2 - 1:
                    # early half-wave for the slow-path m-tiles (rows with
                    # 4-mod-8 DRAM bases serialize on one SDMA engine): start
                    # their serial stream half a group earlier to hide it
                    g0 = b - (GRP // 2 - 1)
                    for mt in (1, 3):
                        mm = 128 if mt < 3 else 127
                        nc.sync.dma_start(
                            out=out_d[
                                g0 : g0 + GRP // 2,
                                mt : mt + 4 * (mm - 1) + 1 : 4,
                                :,
                            ].rearrange("g m k -> m g k"),
                            in_=magmt[mt][:mm, 0 : GRP // 2, :],
                        )
                if b % GRP == GRP - 1:
                    # batch-striped 1028B-descriptor stores on the sync ring
                    # (descriptor stream inner dim = g, 525KB apart in DRAM,
                    # spreads across the SDMA engines)
                    g0 = b - (GRP - 1)
                    for mt in range(4):
                        mm = 128 if mt < 3 else 127  # frame 511 is junk
                        gs = 0 if mt % 2 == 0 else GRP // 2
                        nc.sync.dma_start(
                            out=out_d[
                                g0 + gs : g0 + GRP,
                                mt : mt + 4 * (mm - 1) + 1 : 4,
                                :,
                            ].rearrange("g m k -> m g k"),
                            in_=magmt[mt][:mm, gs:GRP, :],
                        )

    nc.compile()
    _CACHE["nc"] = nc
    return nc


def make_inputs(x, window=None, fourier_matrix=None):
    """Host-side prep: fp16 cast/layout + combined DFT matrix."""
    x = np.asarray(x, dtype=np.float32)
    if window is None or fourier_matrix is None:
        window, fourier_matrix = _default_consts()
    window = np.asarray(window)
    fourier_matrix = np.asarray(fourier_matrix)

    wfm = fourier_matrix.astype(np.complex64) * window.astype(np.float32)[:, None]
    wfm_cat = np.concatenate(
        [wfm.real[:, 0:257], wfm.imag[:, 1:256]], axis=1
    ).astype(np.float16)  # [512, 512]
    wfm_in = np.ascontiguousarray(wfm_cat.reshape(4, 128, N))

    # pre-transposed: x16[b, e, c] = x[b, 128*c + e]
    x16 = np.ascontiguousarray(
        x.astype(np.float16).reshape(B, 1024, 128).transpose(0, 2, 1)
    )
    return x16, wfm_in


def kernel(x, window=None, fourier_matrix=None, **_unused):
    from concourse.bass_utils import run_bass_kernel_spmd

    x16, wfm_in = make_inputs(x, window, fourier_matrix)
    nc = _build()
    in_maps = [
        {"x": x16[i * BPC : (i + 1) * BPC], "wfm": wfm_in} for i in range(NCORES)
    ]
    res = run_bass_kernel_spmd(nc, in_maps, core_ids=list(range(NCORES)))
    return np.concatenate([r["out"] for r in res.results], axis=0)


if __name__ == "__main__":
    rng = np.random.default_rng(0)
    x = rng.standard_normal((B, L)).astype(np.float32)
    out = kernel(x)
    print("out", out.shape, out.dtype, float(out.max()))
